# revision 14
# baseline (speedup 1.0000x reference)
"""Trainium2 Bass kernel for nn_ADSA_31061203484966 (channel-attention dense
transformer block). Pure data-parallel over batch B=8 across 8 NeuronCores.

All five conv groups (qkv, cat, fuse, mlp) run as Winograd F(2x2,3x3) in
fp16 (V/U fp16, PSUM f32).  q,k are produced channel-major, PE-transposed
for the channel-attention Gram matrices (R_q via DVE reduce, R_k via
ones-row matmul); the ds/pointwise/attention composition CW is formed on
device in the Winograd domain from host-transformed ds weights.
"""
import sys

for _p in ("/opt/trn_rl_repo", "/root/.axon_site/_ro/trn_rl_repo"):
    if _p not in sys.path:
        sys.path.append(_p)

import numpy as np
import concourse.bass as bass
import concourse.tile as tile
from concourse import bacc, mybir
from concourse.bass_utils import run_bass_kernel_spmd

f32 = mybir.dt.float32
f32r = mybir.dt.float32r
fp16 = mybir.dt.float16
AF = mybir.ActivationFunctionType
OP = mybir.AluOpType

B, C, H, W = 8, 256, 64, 64
NH, HD = 4, 64
N = H * W                    # 4096
EPS = 1e-5
PADLEN = 4488                # 66*66 padded row-major layout + tail guard
QLEN = 4224                  # unpadded spatial buffers: 4096 + tail guard

_CACHE = {}


def _pad_off(row, dx=0):
    # image pixel (r, c) lives at column 68 + 66*r + c
    return 68 + 66 * row + dx


def _pad_dst(tl, nt):
    off = _pad_off(8 * nt)
    return tl[:, off:off + 528].rearrange("p (r c) -> p r c", c=66)[:, :, 0:64]


def _pad_rhs(tl, nt, dy, dx):
    off = _pad_off(8 * nt + dy, dx)
    return tl[:, off:off + 528].rearrange("p (r c) -> p r c", c=66)[:, :, 0:64]


def _zero_pads(nc, tl, zeros):
    nc.vector.tensor_copy(tl[:, 0:68], zeros[:, 0:68])
    nc.vector.tensor_copy(
        tl[:, 132:132 + 64 * 66].rearrange("p (r c) -> p r c", c=66)[:, :, 0:2],
        zeros[:, 0:128].rearrange("p (r c) -> p r c", c=2))
    nc.vector.tensor_copy(tl[:, 4292:PADLEN], zeros[:, 0:PADLEN - 4292])


# F(2,3) stage-1 row combos: t0 = d0-d2, t1 = d1+d2, t2 = d2-d1, t3 = d1-d3
_ST1 = [(0, 2, OP.subtract), (1, 2, OP.add), (2, 1, OP.subtract),
        (1, 3, OP.subtract)]
# identical combos along columns for stage 2
_ST2 = _ST1


def _col_ap(sc, a, rows):
    """[p, rows, 32tx, 1] AP over a [128, rows*66] stage-1 tile: col 2tx+a."""
    two, off = a % 2, a // 2
    return sc[:, 0:rows * 66].rearrange(
        "p (ty tx two) -> p ty tx two", tx=33, two=2)[
        :, :, off:off + 32, two:two + 1]


def _q_dst(qsp, ch, i, ix):
    """[p, 8ty, 32tx, 1] scatter AP into unpadded [128, QLEN] spatial."""
    base = 1024 * ch + 64 * i
    return qsp[:, base:base + 1024].rearrange(
        "p (ty tx two) -> p ty tx two", tx=64, two=2)[
        :, :, 0:32, ix:ix + 1]


def _v_dst(vp, ch, i, ix):
    """[p, 8ty, 32tx, 1] scatter AP into a padded tile."""
    base = _pad_off(16 * ch + i)
    return vp[:, base:base + 1056].rearrange(
        "p (ty tx two) -> p ty tx two", tx=66, two=2)[
        :, :, 0:32, ix:ix + 1]


def _4d(ap2d, ty=8):
    return ap2d.rearrange("p (ty tx one) -> p ty tx one", ty=ty, one=1)


def _wino_in_full(nc, pool, tag, src, V, ic):
    """Full-image F(2,3) input transform of padded `src` into V[ta][ic]."""
    e1 = nc.gpsimd if ic % 2 == 0 else nc.vector
    e2 = nc.vector if ic % 2 == 0 else nc.gpsimd
    for j1, (ja, jb, op) in enumerate(_ST1):
        sc = pool.tile([128, 2112], fp16, tag=f"{tag}{ic}",
                       name=f"{tag}{ic}_{j1}")
        s = []
        for j in (ja, jb):
            off = 66 * j + 1
            s.append(src[:, off:off + 4224].rearrange(
                "p (ty a) -> p ty a", a=132)[:, :, 0:66])
        e1.tensor_tensor(sc.rearrange("p (ty c) -> p ty c", c=66),
                         s[0], s[1], op)
        for j2, (ca, cb, op2) in enumerate(_ST2):
            dst = V[4 * j1 + j2][ic][:, 0:1024].rearrange(
                "p (ty tx one) -> p ty tx one", tx=32, one=1)
            e2.tensor_tensor(dst, _col_ap(sc, ca, 32), _col_ap(sc, cb, 32),
                             op2)


def _wino_in_chunk(nc, pool, tag, src, Vc, ic, ch):
    """Chunk (8 ty-rows) input transform of padded `src` into Vc[ta][ic]."""
    e1 = nc.gpsimd if ic % 2 == 0 else nc.vector
    e2 = nc.vector if ic % 2 == 0 else nc.gpsimd
    for j1, (ja, jb, op) in enumerate(_ST1):
        sc = pool.tile([128, 528], fp16, tag=f"{tag}{ic}",
                       name=f"{tag}{ic}_{j1}_{ch}")
        s = []
        for j in (ja, jb):
            off = 66 * j + 1 + 1056 * ch
            s.append(src[:, off:off + 1056].rearrange(
                "p (ty a) -> p ty a", a=132)[:, :, 0:66])
        e1.tensor_tensor(sc.rearrange("p (ty c) -> p ty c", c=66),
                         s[0], s[1], op)
        for j2, (ca, cb, op2) in enumerate(_ST2):
            dst = Vc[4 * j1 + j2][ic][:, 0:256].rearrange(
                "p (ty tx one) -> p ty tx one", tx=32, one=1)
            e2.tensor_tensor(dst, _col_ap(sc, ca, 8), _col_ap(sc, cb, 8), op2)


def _wino_oc(nc, psW, pP, stq, name, getU, getV, n_ic, emit):
    """Winograd matmuls + output transform for one (ocT, chunk):
    getU(ta, ic) -> [128,128] lhsT; getV(ta, ic) -> [128,256] rhs;
    emit(i, ix, tmp, Pjc, op2) issues the final stage-2 op."""
    Ps = pP.tile([128, 2048], fp16, tag="P", name=f"P{name}")
    Pr = Ps.rearrange("p (i j x) -> p i j x", i=2, x=256)
    for j2 in range(4):
        ps = psW.tile([128, 1024], f32, tag="mw", name=f"mw{name}_{j2}")
        psr = ps.rearrange("p (j x) -> p j x", x=256)
        for j1 in range(4):
            ta = 4 * j1 + j2
            for ic in range(n_ic):
                nc.tensor.matmul(psr[:, j1], getU(ta, ic), getV(ta, ic),
                                 start=(ic == 0), stop=(ic == n_ic - 1))
        # only one PSUM operand allowed per DVE op: stage M1/M2 via ACT
        cc = stq.tile([128, 512], fp16, tag="cc", name=f"cc{name}_{j2}")
        nc.scalar.copy(cc[:, 0:256], psr[:, 1])
        nc.scalar.copy(cc[:, 256:512], psr[:, 2])
        nc.vector.tensor_tensor(Pr[:, 0, j2], psr[:, 0], cc[:, 0:256], OP.add)
        nc.vector.tensor_tensor(Pr[:, 0, j2], Pr[:, 0, j2], cc[:, 256:512],
                                OP.add)
        nc.vector.tensor_tensor(Pr[:, 1, j2], cc[:, 0:256], cc[:, 256:512],
                                OP.subtract)
        nc.vector.tensor_tensor(Pr[:, 1, j2], Pr[:, 1, j2], psr[:, 3],
                                OP.subtract)
    for i in range(2):
        for ix in range(2):
            if ix == 0:
                ja, jb, jc, op1, op2 = 0, 1, 2, OP.add, OP.add
            else:
                ja, jb, jc, op1, op2 = 1, 2, 3, OP.subtract, OP.subtract
            tmp = stq.tile([128, 256], fp16, tag="y", name=f"y{name}{i}{ix}")
            nc.gpsimd.tensor_tensor(tmp[:], Pr[:, i, ja], Pr[:, i, jb], op1)
            emit(i, ix, tmp, Pr[:, i, jc], op2)


def _build():
    nc = bacc.Bacc("TRN2", target_bir_lowering=False, debug=False, num_devices=8)

    xp_d = nc.dram_tensor("xp", [2, 128, PADLEN], f32, kind="ExternalInput").ap()
    uqkv_d = nc.dram_tensor("uqkv", [16, 2, 6, 128, 128], fp16,
                            kind="ExternalInput").ap()
    dsw16_d = nc.dram_tensor("dsw16", [4, 16, 2, 128, 256], fp16,
                             kind="ExternalInput").ap()
    ufuse_d = nc.dram_tensor("ufuse", [16, 4, 2, 128, 128], fp16,
                             kind="ExternalInput").ap()
    umlp_d = nc.dram_tensor("umlp", [16, 2, 2, 128, 128], fp16,
                            kind="ExternalInput").ap()
    consts_d = nc.dram_tensor("consts", [2, 128, 16], f32, kind="ExternalInput").ap()
    ident_d = nc.dram_tensor("ident", [128, 128], f32, kind="ExternalInput").ap()
    out_d = nc.dram_tensor("out", [C, N], f32, kind="ExternalOutput").ap()

    with tile.TileContext(nc) as tc:
        with tc.tile_pool(name="persist", bufs=1) as persist:
            zeros = persist.tile([128, 264], f32, name="zeros")
            nc.vector.memset(zeros[:], 0.0)
            ones = persist.tile([128, 160], f32, name="ones")
            nc.vector.memset(ones[:], 1.0)
            identf = persist.tile([128, 128], f32, name="identf")
            nc.sync.dma_start(identf[:], ident_d[:])
            ident16 = persist.tile([128, 128], fp16, name="ident16")
            nc.vector.tensor_copy(ident16[:], identf[:])
            ones16 = persist.tile([128, 1], fp16, name="ones16")
            nc.vector.tensor_copy(ones16[:], ones[:, 0:1])
            consts = [persist.tile([128, 16], f32, name=f"consts{m}")
                      for m in range(2)]
            for m in range(2):
                nc.sync.dma_start(consts[m][:], consts_d[m])

            v_pad = [persist.tile([128, PADLEN], fp16, name=f"v_pad{m}")
                     for m in range(2)]
            for m in range(2):
                _zero_pads(nc, v_pad[m], zeros)

            wblk = {}
            for xx in range(4):
                for ch in range(2):
                    t_ = persist.tile([128, 128], fp16, name=f"wblk{xx}_{ch}")
                    nc.vector.tensor_copy(t_[:], zeros[:, 0:128])
                    wblk[(xx, ch)] = t_
            b_sb = [persist.tile([128, 64], f32, name=f"bsb{h}") for h in range(4)]
            nm_rq = [persist.tile([128, 1], f32, name=f"nmrq{h}") for h in range(4)]
            rqc = [persist.tile([128, 1], f32, name=f"rq{t}") for t in range(2)]
            rksb = persist.tile([128, 256], f32, name="rksb")

            # ======== stage A: qkv winograd ========
            with tc.tile_pool(name="pqk", bufs=1) as pqk:
                qsp = [pqk.tile([128, QLEN], fp16, name=f"qsp{t}")
                       for t in range(4)]

                with tc.tile_pool(name="pV", bufs=1) as pV:
                    V = [[pV.tile([128, 1024], fp16, name=f"V{ta}_{ic}")
                          for ic in range(2)] for ta in range(16)]

                    with tc.tile_pool(name="pxp", bufs=1) as pxp, \
                         tc.tile_pool(name="pscr", bufs=2) as pscr:
                        xp = []
                        for m in range(2):
                            t_ = pxp.tile([128, PADLEN], f32, name=f"xp{m}")
                            eng = nc.sync if m == 0 else nc.gpsimd
                            eng.dma_start(t_[:], xp_d[m])
                            xp.append(t_)
                        for ic in range(2):
                            _wino_in_full(nc, pscr, "sx", xp[ic][:], V, ic)

                    with tc.tile_pool(name="wq", bufs=2) as wq, \
                         tc.tile_pool(name="pP", bufs=2) as pP, \
                         tc.tile_pool(name="stq", bufs=4) as stq, \
                         tc.tile_pool(name="psW", bufs=2, space="PSUM") as psW:
                        for ocT in range(6):
                            uw = {}
                            for ta in range(16):
                                for ic in range(2):
                                    w_ = wq.tile([128, 128], fp16,
                                                 tag=f"u{ta}_{ic}",
                                                 name=f"u{ocT}_{ta}_{ic}")
                                    nc.sync.dma_start(w_[:], uqkv_d[ta, ic, ocT])
                                    uw[(ta, ic)] = w_
                            for ch in range(4):
                                if ocT < 4:
                                    def emit(i, ix, tmp, Pjc, op2,
                                             ocT=ocT, ch=ch):
                                        nc.gpsimd.tensor_tensor(
                                            _q_dst(qsp[ocT], ch, i, ix),
                                            _4d(tmp[:]), _4d(Pjc), op2)
                                else:
                                    def emit(i, ix, tmp, Pjc, op2,
                                             m=ocT - 4, ch=ch):
                                        nc.vector.scalar_tensor_tensor(
                                            _v_dst(v_pad[m], ch, i, ix),
                                            _4d(tmp[:]), consts[m][:, 2:3],
                                            _4d(Pjc), OP.add, op2)
                                _wino_oc(
                                    nc, psW, pP, stq, f"a{ocT}_{ch}",
                                    lambda ta, ic: uw[(ta, ic)][:],
                                    lambda ta, ic, ch=ch: V[ta][ic][
                                        :, 256 * ch:256 * ch + 256],
                                    2, emit)

                # ======== stage B: transposes, margins, W formation ========
                with tc.tile_pool(name="pqt", bufs=1) as pqt, \
                     tc.tile_pool(name="psT", bufs=2, space="PSUM") as psT, \
                     tc.tile_pool(name="psS", bufs=1, space="PSUM") as psS:
                    qT = pqt.tile([128, 8192], fp16, name="qT")
                    kT = pqt.tile([128, 8192], fp16, name="kT")
                    for t in range(4):
                        dstT = qT if t < 2 else kT
                        for ch in range(32):
                            pst = psT.tile([128, 128], fp16, tag="tr",
                                           name=f"tr{t}_{ch}")
                            nc.tensor.transpose(
                                pst[:], qsp[t][:, 128 * ch:128 * ch + 128],
                                ident16[:])
                            nc.scalar.copy(
                                dstT[:, 256 * ch + 128 * (t % 2):
                                     256 * ch + 128 * (t % 2) + 128], pst[:])
                    for cT in range(2):
                        nc.vector.tensor_reduce(
                            rqc[cT][:, 0:1], qsp[cT][:, 0:4096],
                            mybir.AxisListType.X, OP.add)
                    psr = psS.tile([128, 256], f32, tag="rk", name="psrk")
                    for ch in range(32):
                        nc.tensor.matmul(
                            psr[0:1, :], ones16[:, 0:1],
                            kT[:, 256 * ch:256 * ch + 256],
                            start=(ch == 0), stop=(ch == 31))
                    nc.scalar.copy(rksb[0:1, :], psr[0:1, :])

                    for h in range(4):
                        pss = psS.tile([128, 64], f32, tag="s", name=f"s{h}")
                        for ch in range(32):
                            nc.tensor.matmul(
                                pss[0:64, :],
                                qT[:, 256 * ch + 64 * h:256 * ch + 64 * h + 64],
                                kT[:, 256 * ch + 64 * h:256 * ch + 64 * h + 64],
                                start=(ch == 0), stop=(ch == 31))
                        nc.vector.tensor_copy(b_sb[h][0:64, :], pss[0:64, :])

                        ch2, p0 = h // 2, 64 * (h % 2)
                        sl = slice(p0, p0 + 64)
                        # W_aa = S
                        nc.vector.tensor_copy(wblk[(0, ch2)][sl, sl],
                                              b_sb[h][0:64, :])
                        # W_ai = R_q - S
                        nc.vector.tensor_scalar(
                            wblk[(2, ch2)][sl, sl], b_sb[h][0:64, :],
                            -1.0, rqc[ch2][sl, 0:1], OP.mult, OP.add)
                        # R_k broadcast across partitions via rank-1 matmul
                        psb = psS.tile([128, 64], f32, tag="rb", name=f"rb{h}")
                        nc.tensor.matmul(
                            psb[0:64, :], ones[0:1, 0:64],
                            rksb[0:1, 64 * h:64 * h + 64],
                            start=True, stop=True)
                        # W_ia = R_k - S
                        nc.vector.tensor_tensor(
                            wblk[(3, ch2)][sl, sl], psb[0:64, :],
                            b_sb[h][0:64, :], OP.subtract)
                        # W_ii = (N - R_q) - W_ia
                        nc.vector.tensor_scalar(
                            nm_rq[h][0:64, 0:1], rqc[ch2][sl, 0:1],
                            -1.0, float(N), OP.mult, OP.add)
                        nc.vector.tensor_scalar(
                            wblk[(1, ch2)][sl, sl],
                            wblk[(3, ch2)][sl, sl],
                            -1.0, nm_rq[h][0:64, 0:1], OP.mult, OP.add)

            # ======== stage C: CW in wino domain, cat winograd ========
            with tc.tile_pool(name="pcat", bufs=1) as pcat:
                cat_pad = [pcat.tile([128, PADLEN], fp16, name=f"cat_pad{i}")
                           for i in range(4)]
                for i in range(4):
                    _zero_pads(nc, cat_pad[i], zeros)

                with tc.tile_pool(name="pcw", bufs=1) as pcw:
                    cw = pcw.tile([128, 16384], fp16, name="cw")
                    with tc.tile_pool(name="wd", bufs=4) as wd, \
                         tc.tile_pool(name="psB", bufs=2, space="PSUM") as psB:
                        for pair in range(2):
                            for ta in range(16):
                                for ic in range(2):
                                    ps = psB.tile([128, 256], f32, tag="mg",
                                                  name=f"cwp{pair}{ta}{ic}")
                                    for xi in range(2):
                                        xx = 2 * pair + xi
                                        w_ = wd.tile([128, 256], fp16,
                                                     tag="dsw",
                                                     name=f"dsw{xx}{ta}{ic}")
                                        nc.sync.dma_start(w_[:],
                                                          dsw16_d[xx, ta, ic])
                                        nc.tensor.matmul(
                                            ps[:], wblk[(xx, ic)][:], w_[:],
                                            start=(xi == 0), stop=(xi == 1))
                                    col = ((pair * 16 + ta) * 2 + ic) * 256
                                    nc.vector.tensor_copy(
                                        cw[:, col:col + 256], ps[:])

                    # cat winograd over v_pad (full-res V)
                    with tc.tile_pool(name="pVv", bufs=1) as pVv, \
                         tc.tile_pool(name="pscv", bufs=2) as pscv, \
                         tc.tile_pool(name="pP2", bufs=2) as pP2, \
                         tc.tile_pool(name="st2", bufs=4) as st2, \
                         tc.tile_pool(name="psW2", bufs=2, space="PSUM") as psW2:
                        Vv = [[pVv.tile([128, 1024], fp16, name=f"Vv{ta}_{ic}")
                               for ic in range(2)] for ta in range(16)]
                        for ic in range(2):
                            _wino_in_full(nc, pscv, "sv", v_pad[ic][:], Vv, ic)
                        for ocT in range(4):
                            pair, o = ocT // 2, ocT % 2
                            for ch in range(4):
                                def emit(i, ix, tmp, Pjc, op2,
                                         oc=ocT, o=o, pair=pair, ch=ch):
                                    nc.vector.scalar_tensor_tensor(
                                        _v_dst(cat_pad[oc], ch, i, ix),
                                        _4d(tmp[:]),
                                        consts[o][:, 3 + pair:4 + pair],
                                        _4d(Pjc), OP.add, op2)
                                _wino_oc(
                                    nc, psW2, pP2, st2, f"c{ocT}_{ch}",
                                    lambda ta, ic, pair=pair, o=o: cw[
                                        :, ((pair * 16 + ta) * 2 + ic) * 256
                                        + 128 * o:
                                        ((pair * 16 + ta) * 2 + ic) * 256
                                        + 128 * o + 128],
                                    lambda ta, ic, ch=ch: Vv[ta][ic][
                                        :, 256 * ch:256 * ch + 256],
                                    2, emit)

                # ======== stage D: fuse + mlp winograd ========
                with tc.tile_pool(name="py2", bufs=1) as py2, \
                     tc.tile_pool(name="stf", bufs=4) as stf:
                    y2_pad = [py2.tile([128, PADLEN], fp16, name=f"y2_pad{m}")
                              for m in range(2)]
                    for m in range(2):
                        _zero_pads(nc, y2_pad[m], zeros)

                    def wino_stage(srcs, n_ic, u_d, pre, nm):
                        """Chunked winograd conv -> unpadded preact tiles."""
                        with tc.tile_pool(name=f"wu{nm}", bufs=1) as wu, \
                             tc.tile_pool(name=f"pVc{nm}", bufs=1) as pVc, \
                             tc.tile_pool(name=f"psc{nm}", bufs=2) as pscn, \
                             tc.tile_pool(name=f"pP{nm}", bufs=2) as pPn, \
                             tc.tile_pool(name=f"st{nm}", bufs=4) as stn, \
                             tc.tile_pool(name=f"psW{nm}", bufs=2,
                                          space="PSUM") as psWn:
                            uw = {}
                            for ta in range(16):
                                for ic in range(n_ic):
                                    for o in range(2):
                                        w_ = wu.tile([128, 128], fp16,
                                                     name=f"wu{nm}{ta}_{ic}_{o}")
                                        nc.sync.dma_start(w_[:], u_d[ta, ic, o])
                                        uw[(ta, ic, o)] = w_
                            for ch in range(4):
                                Vc = [[pVc.tile([128, 256], fp16,
                                                tag=f"Vc{ta}_{ic}",
                                                name=f"Vc{nm}{ta}_{ic}_{ch}")
                                       for ic in range(n_ic)]
                                      for ta in range(16)]
                                for ic in range(n_ic):
                                    _wino_in_chunk(nc, pscn, f"s{nm}",
                                                   srcs[ic][:], Vc, ic, ch)
                                for o in range(2):
                                    def emit(i, ix, tmp, Pjc, op2,
                                             o=o, ch=ch):
                                        nc.gpsimd.tensor_tensor(
                                            _q_dst(pre[o], ch, i, ix),
                                            _4d(tmp[:]), _4d(Pjc), op2)
                                    _wino_oc(
                                        nc, psWn, pPn, stn, f"{nm}{o}_{ch}",
                                        lambda ta, ic, o=o: uw[(ta, ic, o)][:],
                                        lambda ta, ic: Vc[ta][ic][:, 0:256],
                                        n_ic, emit)

                    with tc.tile_pool(name="pprf", bufs=1) as pprf:
                        pre_f = [pprf.tile([128, QLEN], fp16, name=f"pref{o}")
                                 for o in range(2)]
                        wino_stage(cat_pad, 4, ufuse_d, pre_f, "f")
                        # gelu + residual(v) + norm affine -> y2_pad
                        for oc in range(2):
                            for nt in range(8):
                                g1 = stf.tile([128, 512], f32, tag="g1",
                                              name=f"g1{oc}{nt}")
                                nc.scalar.activation(
                                    g1[:],
                                    pre_f[oc][:, 512 * nt:512 * nt + 512],
                                    AF.Gelu_apprx_tanh,
                                    bias=consts[oc][:, 5:6], scale=1.0)
                                g2 = stf.tile([128, 512], f32, tag="g2",
                                              name=f"g2{oc}{nt}")
                                nc.vector.tensor_tensor(
                                    g2[:].rearrange("p (r c) -> p r c", c=64),
                                    g1[:].rearrange("p (r c) -> p r c", c=64),
                                    _pad_rhs(v_pad[oc], nt, 0, 0), OP.add)
                                nc.vector.tensor_scalar(
                                    _pad_dst(y2_pad[oc], nt),
                                    g2[:].rearrange("p (r c) -> p r c", c=64),
                                    consts[oc][:, 6:7], consts[oc][:, 7:8],
                                    OP.mult, OP.add)

                    with tc.tile_pool(name="pprm", bufs=1) as pprm:
                        pre_m = [pprm.tile([128, QLEN], fp16, name=f"prem{o}")
                                 for o in range(2)]
                        wino_stage(y2_pad, 2, umlp_d, pre_m, "m")
                        # gelu + residual(y2) -> out
                        for oc in range(2):
                            for nt in range(8):
                                g1 = stf.tile([128, 512], f32, tag="g1",
                                              name=f"mg1{oc}{nt}")
                                nc.scalar.activation(
                                    g1[:],
                                    pre_m[oc][:, 512 * nt:512 * nt + 512],
                                    AF.Gelu_apprx_tanh,
                                    bias=consts[oc][:, 8:9], scale=1.0)
                                g3 = stf.tile([128, 512], f32, tag="g2",
                                              name=f"mo{oc}{nt}")
                                nc.vector.tensor_tensor(
                                    g3[:].rearrange("p (r c) -> p r c", c=64),
                                    g1[:].rearrange("p (r c) -> p r c", c=64),
                                    _pad_rhs(y2_pad[oc], nt, 0, 0), OP.add)
                                nc.sync.dma_start(
                                    out_d[128 * oc:128 * oc + 128,
                                          512 * nt:512 * nt + 512],
                                    g3[:])

    nc.compile()
    return nc


G2W = np.array([[1, 0, 0], [.5, .5, .5], [.5, -.5, .5], [0, 0, 1]], np.float64)


def _prep(inputs):
    def bn_fold(g, b, m, v):
        s = g.astype(np.float64) / np.sqrt(v.astype(np.float64) + EPS)
        return s, b.astype(np.float64) - m.astype(np.float64) * s

    scale = C ** (-0.5)
    s_qkv, b_qkv = bn_fold(inputs['qkv_g'], inputs['qkv_b'], inputs['qkv_m'], inputs['qkv_v'])
    qkv_w = inputs['qkv_w'].astype(np.float64)
    qkv_wT = (qkv_w * s_qkv[:, :, None, None, None]).transpose(0, 3, 4, 2, 1)
    U = np.einsum('ak,bl,jklio->jabio', G2W, G2W, qkv_wT)
    uqkv = np.zeros((16, 2, 6, 128, 128), np.float16)
    for j in range(3):
        for a in range(4):
            for b in range(4):
                blk = U[j, a, b]
                for icT in range(2):
                    for och in range(2):
                        uqkv[4 * a + b, icT, 2 * j + och] = blk[
                            128 * icT:128 * icT + 128,
                            128 * och:128 * och + 128].astype(np.float16)

    s_ds, b_ds = bn_fold(inputs['ds_g'], inputs['ds_b'], inputs['ds_m'], inputs['ds_v'])
    pw = inputs['pw_w'].astype(np.float64)[:, :, :, 0, 0]
    dw = inputs['dw_w'].astype(np.float64)[:, :, 0, :, :].reshape(4, C, 9)
    dsT = (pw.transpose(0, 2, 1)[:, None, :, :] * dw.transpose(0, 2, 1)[:, :, :, None]
           * s_ds[:, None, None, :]) * scale          # [4, 9, i, o]
    ds9 = dsT.reshape(4, 3, 3, C, C)
    dsWn = np.einsum('ak,bl,xklio->xabio', G2W, G2W, ds9).reshape(4, 16, C, C)
    dsw16 = np.ascontiguousarray(
        dsWn.reshape(4, 16, 2, 128, C).astype(np.float16))

    s_f, b_f = bn_fold(inputs['fuse_g'], inputs['fuse_b'], inputs['fuse_m'], inputs['fuse_v'])
    fuse_wT = (inputs['fuse_w'].astype(np.float64) * s_f[:, None, None, None]
               ).transpose(2, 3, 1, 0).reshape(3, 3, 2 * C, C)
    fWn = np.einsum('ak,bl,klio->abio', G2W, G2W, fuse_wT).reshape(16, 2 * C, C)
    ufuse = np.ascontiguousarray(
        fWn.reshape(16, 4, 128, 2, 128).transpose(0, 1, 3, 2, 4).astype(np.float16))

    s_n, t_n = bn_fold(inputs['norm_g'], inputs['norm_b'], inputs['norm_m'], inputs['norm_v'])
    s_m, b_m = bn_fold(inputs['mlp_g'], inputs['mlp_b'], inputs['mlp_m'], inputs['mlp_v'])
    mlp_wT = (inputs['mlp_w'].astype(np.float64) * s_m[:, None, None, None]
              ).transpose(2, 3, 1, 0).reshape(3, 3, C, C)
    mWn = np.einsum('ak,bl,klio->abio', G2W, G2W, mlp_wT).reshape(16, C, C)
    umlp = np.ascontiguousarray(
        mWn.reshape(16, 2, 128, 2, 128).transpose(0, 1, 3, 2, 4).astype(np.float16))

    consts = np.zeros((2, 128, 16), np.float64)
    cols = [b_qkv[0], b_qkv[1], b_qkv[2],
            b_ds[0] + b_ds[1], b_ds[2] + b_ds[3],
            b_f, s_n, t_n, b_m]
    for ci, v in enumerate(cols):
        consts[0, :, ci] = v[0:128]
        consts[1, :, ci] = v[128:256]
    consts = consts.astype(np.float32)

    ident = np.eye(128, dtype=np.float32)
    return {"uqkv": uqkv, "dsw16": dsw16, "ufuse": ufuse, "umlp": umlp,
            "consts": consts, "ident": ident}


def _host_pad(xb):
    """[C, H, W] -> [2, 128, PADLEN] padded-66 layout, image at 68+66r+c."""
    xp = np.zeros((2, 128, PADLEN), np.float32)
    xp[:, :, 68:68 + 64 * 66].reshape(2, 128, 64, 66)[:, :, :, 0:64] = \
        xb.reshape(2, 128, H, W)
    return xp


def make_in_maps(inputs):
    shared = _prep(inputs)
    x = inputs['x'].astype(np.float32)
    return [{"xp": _host_pad(x[b]), **shared} for b in range(B)]


def kernel(**inputs):
    inputs = {k: np.asarray(v) for k, v in inputs.items()}
    if "nc" not in _CACHE:
        _CACHE["nc"] = _build()
    nc = _CACHE["nc"]
    in_maps = make_in_maps(inputs)
    res = run_bass_kernel_spmd(nc, in_maps, core_ids=list(range(8)))
    out = np.stack([res.results[b]["out"] for b in range(B)])
    return out.reshape(B, C, H, W).astype(np.float32)


# revision 33
# speedup vs baseline: 1.4115x; 1.4115x over previous
"""Trainium2 Bass kernel for nn_ADSA_31061203484966 (channel-attention dense
transformer block). Pure data-parallel over batch B=8 across 8 NeuronCores.

All five conv groups run as Winograd F(2x2,3x3) in fp16 (V/U fp16, PSUM f32).
Spatial buffers are COLUMN-DEINTERLEAVED (within each 66-wide padded row band,
even/odd source columns are grouped) so every transform op is inner-contiguous;
the host permutes x on the way in and un-permutes the output.  q,k are
channel-major, PE-transposed for the channel-attention Gram matrices; CW is
formed on device in the Winograd domain.
"""
import sys

for _p in ("/opt/trn_rl_repo", "/root/.axon_site/_ro/trn_rl_repo"):
    if _p not in sys.path:
        sys.path.append(_p)

import numpy as np
import concourse.bass as bass
import concourse.tile as tile
from concourse import bacc, mybir
from concourse.bass_utils import run_bass_kernel_spmd

f32 = mybir.dt.float32
f32r = mybir.dt.float32r
fp16 = mybir.dt.float16
AF = mybir.ActivationFunctionType
OP = mybir.AluOpType

B, C, H, W = 8, 256, 64, 64
NH, HD = 4, 64
N = H * W                    # 4096
EPS = 1e-5
PADLEN = 4488                # 66*66 padded row-major layout + tail guard
QLEN = 4224                  # unpadded spatial buffers: 4096 + tail guard

# deinterleaved band: idx0 = pad(cc0); idx 1..32 = odd c (c=2k+1); idx 33..64
# = even c (c=2k); idx65 = pad(c=64).  col(a) of a winograd tile tx reads
# band[COFF[a] + tx]; output pixel c = 2tx+ix writes band[VOFF[ix] + tx]
# (offsets relative to _pad_off = band_start+1).
COFF = [0, 33, 1, 34]        # cc = 2tx + a, offsets relative to band start
VOFF = [32, 0]               # relative to _pad_off(row)
QOFF = [32, 0]               # unpadded rows: [odd c | even c], matches _CPOS

_CACHE = {}
DEBUG_TAPS = False


def _pad_off(row, dx=0):
    return 68 + 66 * row + dx


def _pad_dst(tl, nt):
    off = _pad_off(8 * nt)
    return tl[:, off:off + 528].rearrange("p (r c) -> p r c", c=66)[:, :, 0:64]


def _pad_rhs(tl, nt, dy, dx):
    off = _pad_off(8 * nt + dy, dx)
    return tl[:, off:off + 528].rearrange("p (r c) -> p r c", c=66)[:, :, 0:64]


# F(2,3) stage-1 row combos: t0 = d0-d2, t1 = d1+d2, t2 = d2-d1, t3 = d1-d3
_ST1 = [(0, 2, OP.subtract), (1, 2, OP.add), (2, 1, OP.subtract),
        (1, 3, OP.subtract)]
_ST2 = _ST1


def _band_col(sc, a, rows):
    """[p, rows, 32] contiguous-inner AP over a [128, rows*66] stage-1 tile."""
    c0 = COFF[a]
    return sc[:, 0:rows * 66].rearrange(
        "p (ty c) -> p ty c", c=66)[:, :, c0:c0 + 32]


def _q_dst(qsp, ch, i, ix, R=16):
    """[p, R ty, 32] AP into unpadded [128, QLEN] deinterleaved spatial."""
    start = 64 * (2 * R * ch + i) + QOFF[ix]
    return qsp[:, start:start + 128 * R].rearrange(
        "p (ty a) -> p ty a", a=128)[:, :, 0:32]


def _v_dst(vp, ch, i, ix, R=16):
    """[p, R ty, 32] AP into a padded deinterleaved tile."""
    start = _pad_off(2 * R * ch + i) + VOFF[ix]
    return vp[:, start:start + 132 * R].rearrange(
        "p (ty a) -> p ty a", a=132)[:, :, 0:32]


def _wino_in_full(nc, pool, tag, src, V, ic):
    """Full-image F(2,3) input transform of padded `src` into V[ta][ic]."""
    e1 = nc.gpsimd if ic % 2 == 0 else nc.vector
    e2 = nc.vector if ic % 2 == 0 else nc.gpsimd
    for j1, (ja, jb, op) in enumerate(_ST1):
        sc = pool.tile([128, 2112], fp16, tag=f"{tag}{ic}",
                       name=f"{tag}{ic}_{j1}")
        s = []
        for j in (ja, jb):
            off = 66 * j + 1
            s.append(src[:, off:off + 4224].rearrange(
                "p (ty a) -> p ty a", a=132)[:, :, 0:66])
        e1.tensor_tensor(sc.rearrange("p (ty c) -> p ty c", c=66),
                         s[0], s[1], op)
        for j2, (ca, cb, op2) in enumerate(_ST2):
            dst = V[4 * j1 + j2][ic][:, 0:1024].rearrange(
                "p (ty tx) -> p ty tx", tx=32)
            e2.tensor_tensor(dst, _band_col(sc, ca, 32),
                             _band_col(sc, cb, 32), op2)


def _wino_in_chunk(nc, pool, tag, src, Vc, ic, ch, R=16):
    """R tile-rows input transform into Vc[ta][ic] [128, 32*R]."""
    e1 = nc.gpsimd if ic % 2 == 0 else nc.vector
    e2 = nc.vector if ic % 2 == 0 else nc.gpsimd
    for j1, (ja, jb, op) in enumerate(_ST1):
        sc = pool.tile([128, 66 * R], fp16, tag=f"{tag}{ic}",
                       name=f"{tag}{ic}_{j1}_{ch}")
        s = []
        for j in (ja, jb):
            off = 66 * j + 1 + 132 * R * ch
            s.append(src[:, off:off + 132 * R].rearrange(
                "p (ty a) -> p ty a", a=132)[:, :, 0:66])
        e1.tensor_tensor(sc.rearrange("p (ty c) -> p ty c", c=66),
                         s[0], s[1], op)
        for j2, (ca, cb, op2) in enumerate(_ST2):
            dst = Vc[4 * j1 + j2][ic][:, 0:32 * R].rearrange(
                "p (ty tx) -> p ty tx", tx=32)
            e2.tensor_tensor(dst, _band_col(sc, ca, R),
                             _band_col(sc, cb, R), op2)


def _wino_oc(nc, psW, pP, stq, name, getU, getV, n_ic, emit, R=16):
    """Winograd matmuls + output transform for one (ocT, chunk of R tile-rows):
    getU(ta, ic) -> [128,128] lhsT; getV(ta, ic) -> [128,32*R] rhs;
    emit(i, ix, tmp, Pjc, op2) issues the final stage-2 op (dst [p,R,32])."""
    Tw = 32 * R
    Ps = pP.tile([128, 8 * Tw], fp16, tag="P", name=f"P{name}")
    Pr = Ps.rearrange("p (i j x) -> p i j x", i=2, x=Tw)
    for j2 in range(4):
        ps = psW.tile([128, 4 * Tw], f32, tag="mw", name=f"mw{name}_{j2}")
        psr = ps.rearrange("p (j x) -> p j x", x=Tw)
        for j1 in range(4):
            ta = 4 * j1 + j2
            for ic in range(n_ic):
                nc.tensor.matmul(psr[:, j1], getU(ta, ic), getV(ta, ic),
                                 start=(ic == 0), stop=(ic == n_ic - 1))
        # only one PSUM operand allowed per DVE op: stage M1/M2 via ACT
        cc = stq.tile([128, 2 * Tw], fp16, tag="cc", name=f"cc{name}_{j2}")
        nc.scalar.copy(cc[:, 0:Tw], psr[:, 1])
        nc.scalar.copy(cc[:, Tw:2 * Tw], psr[:, 2])
        nc.vector.tensor_tensor(Pr[:, 0, j2], psr[:, 0], cc[:, 0:Tw], OP.add)
        nc.vector.tensor_tensor(Pr[:, 0, j2], Pr[:, 0, j2], cc[:, Tw:2 * Tw],
                                OP.add)
        nc.vector.tensor_tensor(Pr[:, 1, j2], cc[:, 0:Tw], cc[:, Tw:2 * Tw],
                                OP.subtract)
        nc.vector.tensor_tensor(Pr[:, 1, j2], Pr[:, 1, j2], psr[:, 3],
                                OP.subtract)
    for i in range(2):
        for ix in range(2):
            if ix == 0:
                ja, jb, jc, op1, op2 = 0, 1, 2, OP.add, OP.add
            else:
                ja, jb, jc, op1, op2 = 1, 2, 3, OP.subtract, OP.subtract
            tmp = stq.tile([128, Tw], fp16, tag="y", name=f"y{name}{i}{ix}")
            nc.gpsimd.tensor_tensor(tmp[:], Pr[:, i, ja], Pr[:, i, jb], op1)
            emit(i, ix, tmp, Pr[:, i, jc], op2)


def _3d(ap, ty=None):
    return ap.rearrange("p (ty tx) -> p ty tx", tx=32)


def _build():
    nc = bacc.Bacc("TRN2", target_bir_lowering=False, debug=False, num_devices=8)

    xp_d = nc.dram_tensor("xp", [2, 128, PADLEN], f32, kind="ExternalInput").ap()
    uqkv_d = nc.dram_tensor("uqkv", [6, 128, 4096], fp16,
                            kind="ExternalInput").ap()
    dsw16_d = nc.dram_tensor("dsw16", [2, 128, 16384], fp16,
                             kind="ExternalInput").ap()
    ufuse_d = nc.dram_tensor("ufuse", [2, 128, 8192], fp16,
                             kind="ExternalInput").ap()
    umlp_d = nc.dram_tensor("umlp", [2, 128, 4096], fp16,
                            kind="ExternalInput").ap()
    consts_d = nc.dram_tensor("consts", [2, 128, 16], f32, kind="ExternalInput").ap()
    ident_d = nc.dram_tensor("ident", [128, 128], f32, kind="ExternalInput").ap()
    out_d = nc.dram_tensor("out", [C, N], f32, kind="ExternalOutput").ap()
    if DEBUG_TAPS:
        dbg_d = nc.dram_tensor("dbg", [8, 128, PADLEN], f32,
                               kind="ExternalOutput").ap()

    with tile.TileContext(nc) as tc:
        with tc.tile_pool(name="persist", bufs=1) as persist:
            zeros = persist.tile([128, 264], f32, name="zeros")
            nc.vector.memset(zeros[:], 0.0)
            ones = persist.tile([128, 160], f32, name="ones")
            nc.vector.memset(ones[:], 1.0)
            identf = persist.tile([128, 128], f32, name="identf")
            nc.sync.dma_start(identf[:], ident_d[:])
            ident16 = persist.tile([128, 128], fp16, name="ident16")
            nc.vector.tensor_copy(ident16[:], identf[:])
            ones16 = persist.tile([128, 1], fp16, name="ones16")
            nc.vector.tensor_copy(ones16[:], ones[:, 0:1])
            consts = [persist.tile([128, 16], f32, name=f"consts{m}")
                      for m in range(2)]
            for m in range(2):
                nc.sync.dma_start(consts[m][:], consts_d[m])

            v_pad = [persist.tile([128, PADLEN], fp16, name=f"v_pad{m}")
                     for m in range(2)]
            for m in range(2):
                nc.vector.memset(v_pad[m][:], 0.0)

            wblk = {}
            for xx in range(4):
                for ch in range(2):
                    t_ = persist.tile([128, 128], fp16, name=f"wblk{xx}_{ch}")
                    nc.vector.tensor_copy(t_[:], zeros[:, 0:128])
                    wblk[(xx, ch)] = t_
            b_sb = [persist.tile([128, 64], f32, name=f"bsb{h}") for h in range(4)]
            nm_rq = [persist.tile([128, 1], f32, name=f"nmrq{h}") for h in range(4)]
            rqc = [persist.tile([128, 1], f32, name=f"rq{t}") for t in range(2)]
            rksb = persist.tile([128, 256], f32, name="rksb")

            # ======== stage A: qkv winograd ========
            with tc.tile_pool(name="pqk", bufs=1) as pqk:
                qsp = [pqk.tile([128, QLEN], fp16, name=f"qsp{t}")
                       for t in range(4)]

                with tc.tile_pool(name="pV", bufs=1) as pV:
                    V = [[pV.tile([128, 1024], fp16, name=f"V{ta}_{ic}")
                          for ic in range(2)] for ta in range(16)]

                    with tc.tile_pool(name="pxp", bufs=1) as pxp, \
                         tc.tile_pool(name="pscr", bufs=2) as pscr:
                        xp = []
                        for m in range(2):
                            t_ = pxp.tile([128, PADLEN], f32, name=f"xp{m}")
                            eng = nc.sync if m == 0 else nc.gpsimd
                            eng.dma_start(t_[:], xp_d[m])
                            xp.append(t_)
                        for ic in range(2):
                            _wino_in_full(nc, pscr, "sx", xp[ic][:], V, ic)

                    with tc.tile_pool(name="wq", bufs=2) as wq, \
                         tc.tile_pool(name="pP", bufs=2) as pP, \
                         tc.tile_pool(name="stq", bufs=4) as stq, \
                         tc.tile_pool(name="psW", bufs=2, space="PSUM") as psW:
                        for ocT in range(6):
                            ua = wq.tile([128, 4096], fp16, tag="u",
                                         name=f"u{ocT}")
                            nc.sync.dma_start(ua[:], uqkv_d[ocT])
                            for ch in range(2):
                                if ocT < 4:
                                    def emit(i, ix, tmp, Pjc, op2,
                                             ocT=ocT, ch=ch):
                                        nc.gpsimd.tensor_tensor(
                                            _q_dst(qsp[ocT], ch, i, ix),
                                            _3d(tmp[:]), _3d(Pjc), op2)
                                else:
                                    def emit(i, ix, tmp, Pjc, op2,
                                             m=ocT - 4, ch=ch):
                                        nc.vector.scalar_tensor_tensor(
                                            _v_dst(v_pad[m], ch, i, ix),
                                            _3d(tmp[:]), consts[m][:, 2:3],
                                            _3d(Pjc), OP.add, op2)
                                _wino_oc(
                                    nc, psW, pP, stq, f"a{ocT}_{ch}",
                                    lambda ta, ic, ua=ua: ua[
                                        :, (ta * 2 + ic) * 128:
                                        (ta * 2 + ic) * 128 + 128],
                                    lambda ta, ic, ch=ch: V[ta][ic][
                                        :, 512 * ch:512 * ch + 512],
                                    2, emit)

                # ======== stage B: transposes, margins, W formation ========
                with tc.tile_pool(name="pqt", bufs=1) as pqt, \
                     tc.tile_pool(name="psT", bufs=2, space="PSUM") as psT, \
                     tc.tile_pool(name="psS", bufs=1, space="PSUM") as psS:
                    qT = pqt.tile([128, 8192], fp16, name="qT")
                    kT = pqt.tile([128, 8192], fp16, name="kT")
                    for t in range(4):
                        dstT = qT if t < 2 else kT
                        for ch in range(32):
                            pst = psT.tile([128, 128], fp16, tag="tr",
                                           name=f"tr{t}_{ch}")
                            nc.tensor.transpose(
                                pst[:], qsp[t][:, 128 * ch:128 * ch + 128],
                                ident16[:])
                            nc.scalar.copy(
                                dstT[:, 256 * ch + 128 * (t % 2):
                                     256 * ch + 128 * (t % 2) + 128], pst[:])
                    for cT in range(2):
                        nc.vector.tensor_reduce(
                            rqc[cT][:, 0:1], qsp[cT][:, 0:4096],
                            mybir.AxisListType.X, OP.add)
                    psr = psS.tile([128, 256], f32, tag="rk", name="psrk")
                    for ch in range(32):
                        nc.tensor.matmul(
                            psr[0:1, :], ones16[:, 0:1],
                            kT[:, 256 * ch:256 * ch + 256],
                            start=(ch == 0), stop=(ch == 31))
                    nc.scalar.copy(rksb[0:1, :], psr[0:1, :])

                    for h in range(4):
                        pss = psS.tile([128, 64], f32, tag="s", name=f"s{h}")
                        for ch in range(32):
                            nc.tensor.matmul(
                                pss[0:64, :],
                                qT[:, 256 * ch + 64 * h:256 * ch + 64 * h + 64],
                                kT[:, 256 * ch + 64 * h:256 * ch + 64 * h + 64],
                                start=(ch == 0), stop=(ch == 31))
                        nc.vector.tensor_copy(b_sb[h][0:64, :], pss[0:64, :])

                        ch2, p0 = h // 2, 64 * (h % 2)
                        sl = slice(p0, p0 + 64)
                        nc.vector.tensor_copy(wblk[(0, ch2)][sl, sl],
                                              b_sb[h][0:64, :])
                        nc.vector.tensor_scalar(
                            wblk[(2, ch2)][sl, sl], b_sb[h][0:64, :],
                            -1.0, rqc[ch2][sl, 0:1], OP.mult, OP.add)
                        psb = psS.tile([128, 64], f32, tag="rb", name=f"rb{h}")
                        nc.tensor.matmul(
                            psb[0:64, :], ones[0:1, 0:64],
                            rksb[0:1, 64 * h:64 * h + 64],
                            start=True, stop=True)
                        nc.vector.tensor_tensor(
                            wblk[(3, ch2)][sl, sl], psb[0:64, :],
                            b_sb[h][0:64, :], OP.subtract)
                        nc.vector.tensor_scalar(
                            nm_rq[h][0:64, 0:1], rqc[ch2][sl, 0:1],
                            -1.0, float(N), OP.mult, OP.add)
                        nc.vector.tensor_scalar(
                            wblk[(1, ch2)][sl, sl],
                            wblk[(3, ch2)][sl, sl],
                            -1.0, nm_rq[h][0:64, 0:1], OP.mult, OP.add)

            if DEBUG_TAPS:
                nc.gpsimd.dma_start(dbg_d[0][:, 0:4096], qsp[0][:, 0:4096])
                nc.gpsimd.dma_start(dbg_d[1][:, 0:PADLEN], v_pad[0][:])
            # ======== stage C: CW in wino domain, cat winograd ========
            with tc.tile_pool(name="pcat", bufs=1) as pcat:
                cat_pad = [pcat.tile([128, PADLEN], fp16, name=f"cat_pad{i}")
                           for i in range(4)]
                for i in range(4):
                    nc.vector.memset(cat_pad[i][:], 0.0)

                with tc.tile_pool(name="pcw", bufs=1) as pcw:
                    cw = pcw.tile([128, 16384], fp16, name="cw")
                    with tc.tile_pool(name="wd", bufs=2) as wd, \
                         tc.tile_pool(name="psB", bufs=2, space="PSUM") as psB:
                        for pair in range(2):
                            da = wd.tile([128, 16384], fp16, tag="dsw",
                                         name=f"dsw{pair}")
                            nc.sync.dma_start(da[:], dsw16_d[pair])
                            for ta in range(16):
                                for ic in range(2):
                                    ps = psB.tile([128, 256], f32, tag="mg",
                                                  name=f"cwp{pair}{ta}{ic}")
                                    for xi in range(2):
                                        col = (((ta * 2 + ic) * 2 + xi) * 256)
                                        nc.tensor.matmul(
                                            ps[:], wblk[(2 * pair + xi, ic)][:],
                                            da[:, col:col + 256],
                                            start=(xi == 0), stop=(xi == 1))
                                    col = ((pair * 16 + ta) * 2 + ic) * 256
                                    nc.vector.tensor_copy(
                                        cw[:, col:col + 256], ps[:])

                    with tc.tile_pool(name="pVv", bufs=1) as pVv, \
                         tc.tile_pool(name="pscv", bufs=1) as pscv, \
                         tc.tile_pool(name="pP2", bufs=2) as pP2, \
                         tc.tile_pool(name="st2", bufs=4) as st2, \
                         tc.tile_pool(name="psW2", bufs=2, space="PSUM") as psW2:
                        Vv = [[pVv.tile([128, 1024], fp16, name=f"Vv{ta}_{ic}")
                               for ic in range(2)] for ta in range(16)]
                        for ic in range(2):
                            _wino_in_full(nc, pscv, "sv", v_pad[ic][:], Vv, ic)
                        for ocT in range(4):
                            pair, o = ocT // 2, ocT % 2
                            for ch in range(2):
                                def emit(i, ix, tmp, Pjc, op2,
                                         oc=ocT, o=o, pair=pair, ch=ch):
                                    nc.vector.scalar_tensor_tensor(
                                        _v_dst(cat_pad[oc], ch, i, ix),
                                        _3d(tmp[:]),
                                        consts[o][:, 3 + pair:4 + pair],
                                        _3d(Pjc), OP.add, op2)
                                _wino_oc(
                                    nc, psW2, pP2, st2, f"c{ocT}_{ch}",
                                    lambda ta, ic, pair=pair, o=o: cw[
                                        :, ((pair * 16 + ta) * 2 + ic) * 256
                                        + 128 * o:
                                        ((pair * 16 + ta) * 2 + ic) * 256
                                        + 128 * o + 128],
                                    lambda ta, ic, ch=ch: Vv[ta][ic][
                                        :, 512 * ch:512 * ch + 512],
                                    2, emit)

                if DEBUG_TAPS:
                    nc.gpsimd.dma_start(dbg_d[2][:, 0:PADLEN], cat_pad[0][:])
                    nc.gpsimd.dma_start(dbg_d[3][:, 0:128], wblk[(0, 0)][:])
                    nc.gpsimd.dma_start(dbg_d[4][:, 0:128], wblk[(1, 0)][:])
                # ======== stage D: fuse + mlp winograd ========
                with tc.tile_pool(name="py2", bufs=1) as py2, \
                     tc.tile_pool(name="stf", bufs=2) as stf:
                    y2_pad = [py2.tile([128, PADLEN], fp16, name=f"y2_pad{m}")
                              for m in range(2)]
                    for m in range(2):
                        nc.vector.memset(y2_pad[m][:], 0.0)

                    def wino_stage(srcs, n_ic, u_d, usz, pre, nm, R=16):
                        Tw = 32 * R
                        with tc.tile_pool(name=f"wu{nm}", bufs=1) as wu, \
                             tc.tile_pool(name=f"pVc{nm}", bufs=1) as pVc, \
                             tc.tile_pool(name=f"psc{nm}", bufs=1) as pscn, \
                             tc.tile_pool(name=f"pP{nm}", bufs=2) as pPn, \
                             tc.tile_pool(name=f"st{nm}", bufs=2) as stn, \
                             tc.tile_pool(name=f"psW{nm}", bufs=2,
                                          space="PSUM") as psWn:
                            uas = []
                            for o in range(2):
                                ua = wu.tile([128, usz], fp16, tag=f"u{o}",
                                             name=f"wu{nm}{o}")
                                nc.sync.dma_start(ua[:], u_d[o])
                                uas.append(ua)
                            for ch in range(32 // R):
                                Vc = [[pVc.tile([128, Tw], fp16,
                                                tag=f"Vc{ta}_{ic}",
                                                name=f"Vc{nm}{ta}_{ic}_{ch}")
                                       for ic in range(n_ic)]
                                      for ta in range(16)]
                                for ic in range(n_ic):
                                    _wino_in_chunk(nc, pscn, f"s{nm}",
                                                   srcs[ic][:], Vc, ic, ch, R)
                                for o in range(2):
                                    ua = uas[o]
                                    def emit(i, ix, tmp, Pjc, op2,
                                             o=o, ch=ch):
                                        nc.gpsimd.tensor_tensor(
                                            _q_dst(pre[o], ch, i, ix, R),
                                            _3d(tmp[:]), _3d(Pjc), op2)
                                    _wino_oc(
                                        nc, psWn, pPn, stn, f"{nm}{o}_{ch}",
                                        lambda ta, ic, ua=ua: ua[
                                            :, (ta * n_ic + ic) * 128:
                                            (ta * n_ic + ic) * 128 + 128],
                                        lambda ta, ic: Vc[ta][ic][:, 0:Tw],
                                        n_ic, emit, R)

                    with tc.tile_pool(name="pprf", bufs=1) as pprf:
                        pre_f = [pprf.tile([128, QLEN], fp16, name=f"pref{o}")
                                 for o in range(2)]
                        wino_stage(cat_pad, 4, ufuse_d, 8192, pre_f, "f", R=8)
                        if DEBUG_TAPS:
                            nc.gpsimd.dma_start(dbg_d[5][:, 0:4096], pre_f[0][:, 0:4096])
                        for oc in range(2):
                            for nt in range(8):
                                g1 = stf.tile([128, 512], f32, tag="g1",
                                              name=f"g1{oc}{nt}")
                                nc.scalar.activation(
                                    g1[:],
                                    pre_f[oc][:, 512 * nt:512 * nt + 512],
                                    AF.Gelu_apprx_tanh,
                                    bias=consts[oc][:, 5:6], scale=1.0)
                                g2 = stf.tile([128, 512], f32, tag="g2",
                                              name=f"g2{oc}{nt}")
                                nc.vector.tensor_tensor(
                                    g2[:].rearrange("p (r c) -> p r c", c=64),
                                    g1[:].rearrange("p (r c) -> p r c", c=64),
                                    _pad_rhs(v_pad[oc], nt, 0, 0), OP.add)
                                nc.vector.tensor_scalar(
                                    _pad_dst(y2_pad[oc], nt),
                                    g2[:].rearrange("p (r c) -> p r c", c=64),
                                    consts[oc][:, 6:7], consts[oc][:, 7:8],
                                    OP.mult, OP.add)

                    with tc.tile_pool(name="pprm", bufs=1) as pprm:
                        pre_m = [pprm.tile([128, QLEN], fp16, name=f"prem{o}")
                                 for o in range(2)]
                        if DEBUG_TAPS:
                            nc.gpsimd.dma_start(dbg_d[6][:, 0:PADLEN], y2_pad[0][:])
                        wino_stage(y2_pad, 2, umlp_d, 4096, pre_m, "m")
                        for oc in range(2):
                            for nt in range(8):
                                g1 = stf.tile([128, 512], f32, tag="g1",
                                              name=f"mg1{oc}{nt}")
                                nc.scalar.activation(
                                    g1[:],
                                    pre_m[oc][:, 512 * nt:512 * nt + 512],
                                    AF.Gelu_apprx_tanh,
                                    bias=consts[oc][:, 8:9], scale=1.0)
                                g3 = stf.tile([128, 512], f32, tag="g2",
                                              name=f"mo{oc}{nt}")
                                nc.vector.tensor_tensor(
                                    g3[:].rearrange("p (r c) -> p r c", c=64),
                                    g1[:].rearrange("p (r c) -> p r c", c=64),
                                    _pad_rhs(y2_pad[oc], nt, 0, 0), OP.add)
                                nc.sync.dma_start(
                                    out_d[128 * oc:128 * oc + 128,
                                          512 * nt:512 * nt + 512],
                                    g3[:])

    nc.compile()
    return nc


G2W = np.array([[1, 0, 0], [.5, .5, .5], [.5, -.5, .5], [0, 0, 1]], np.float64)

# band-position of image column c in the deinterleaved layout, relative to
# _pad_off(row) (flat band idx 1..64): odd c -> (c-1)//2, even c -> 32 + c//2
_CPOS = np.array([(c - 1) // 2 if c % 2 else 32 + c // 2
                  for c in range(64)], np.int64)


def _prep(inputs):
    def bn_fold(g, b, m, v):
        s = g.astype(np.float64) / np.sqrt(v.astype(np.float64) + EPS)
        return s, b.astype(np.float64) - m.astype(np.float64) * s

    scale = C ** (-0.5)
    s_qkv, b_qkv = bn_fold(inputs['qkv_g'], inputs['qkv_b'], inputs['qkv_m'], inputs['qkv_v'])
    qkv_w = inputs['qkv_w'].astype(np.float64)
    qkv_wT = (qkv_w * s_qkv[:, :, None, None, None]).transpose(0, 3, 4, 2, 1)
    U = np.einsum('ak,bl,jklio->jabio', G2W, G2W, qkv_wT)  # [3,4,4,Cin,Cout]
    # device layout [6 ocT, 128 p(icT-part), 16 ta, 2 icT, 128]
    uqkv = np.zeros((6, 128, 16, 2, 128), np.float16)
    for j in range(3):
        for a in range(4):
            for b in range(4):
                blk = U[j, a, b]
                for icT in range(2):
                    for och in range(2):
                        uqkv[2 * j + och, :, 4 * a + b, icT] = blk[
                            128 * icT:128 * icT + 128,
                            128 * och:128 * och + 128].astype(np.float16)
    uqkv = uqkv.reshape(6, 128, 4096)

    s_ds, b_ds = bn_fold(inputs['ds_g'], inputs['ds_b'], inputs['ds_m'], inputs['ds_v'])
    pw = inputs['pw_w'].astype(np.float64)[:, :, :, 0, 0]
    dw = inputs['dw_w'].astype(np.float64)[:, :, 0, :, :].reshape(4, C, 9)
    dsT = (pw.transpose(0, 2, 1)[:, None, :, :] * dw.transpose(0, 2, 1)[:, :, :, None]
           * s_ds[:, None, None, :]) * scale          # [4, 9, i, o]
    ds9 = dsT.reshape(4, 3, 3, C, C)
    dsWn = np.einsum('ak,bl,xklio->xabio', G2W, G2W, ds9).reshape(4, 16, C, C)
    # device: [2 pair, 128 p(i-part), 16 ta, 2 icT, 2 xi, 256 o]
    dsw16 = np.zeros((2, 128, 16, 2, 2, 256), np.float16)
    for pair in range(2):
        for xi in range(2):
            for icT in range(2):
                dsw16[pair, :, :, icT, xi, :] = dsWn[
                    2 * pair + xi, :, 128 * icT:128 * icT + 128, :
                ].transpose(1, 0, 2).astype(np.float16)
    dsw16 = dsw16.reshape(2, 128, 16384)

    s_f, b_f = bn_fold(inputs['fuse_g'], inputs['fuse_b'], inputs['fuse_m'], inputs['fuse_v'])
    fuse_wT = (inputs['fuse_w'].astype(np.float64) * s_f[:, None, None, None]
               ).transpose(2, 3, 1, 0).reshape(3, 3, 2 * C, C)
    fWn = np.einsum('ak,bl,klio->abio', G2W, G2W, fuse_wT).reshape(16, 2 * C, C)
    # [2 ocT, 128 p, 16 ta, 4 icT, 128]
    ufuse = np.zeros((2, 128, 16, 4, 128), np.float16)
    for icT in range(4):
        for o in range(2):
            ufuse[o, :, :, icT, :] = fWn[
                :, 128 * icT:128 * icT + 128, 128 * o:128 * o + 128
            ].transpose(1, 0, 2).astype(np.float16)
    ufuse = ufuse.reshape(2, 128, 8192)

    s_n, t_n = bn_fold(inputs['norm_g'], inputs['norm_b'], inputs['norm_m'], inputs['norm_v'])
    s_m, b_m = bn_fold(inputs['mlp_g'], inputs['mlp_b'], inputs['mlp_m'], inputs['mlp_v'])
    mlp_wT = (inputs['mlp_w'].astype(np.float64) * s_m[:, None, None, None]
              ).transpose(2, 3, 1, 0).reshape(3, 3, C, C)
    mWn = np.einsum('ak,bl,klio->abio', G2W, G2W, mlp_wT).reshape(16, C, C)
    umlp = np.zeros((2, 128, 16, 2, 128), np.float16)
    for icT in range(2):
        for o in range(2):
            umlp[o, :, :, icT, :] = mWn[
                :, 128 * icT:128 * icT + 128, 128 * o:128 * o + 128
            ].transpose(1, 0, 2).astype(np.float16)
    umlp = umlp.reshape(2, 128, 4096)

    consts = np.zeros((2, 128, 16), np.float64)
    cols = [b_qkv[0], b_qkv[1], b_qkv[2],
            b_ds[0] + b_ds[1], b_ds[2] + b_ds[3],
            b_f, s_n, t_n, b_m]
    for ci, v in enumerate(cols):
        consts[0, :, ci] = v[0:128]
        consts[1, :, ci] = v[128:256]
    consts = consts.astype(np.float32)

    ident = np.eye(128, dtype=np.float32)
    return {"uqkv": uqkv, "dsw16": dsw16, "ufuse": ufuse, "umlp": umlp,
            "consts": consts, "ident": ident}


def _host_pad(xb):
    """[C, H, W] -> [2, 128, PADLEN] deinterleaved padded-66 layout."""
    xp = np.zeros((2, 128, PADLEN), np.float32)
    xr = xb.reshape(2, 128, H, W)
    rows = np.arange(H)
    pos = (68 + 66 * rows[:, None] + _CPOS[None, :]).ravel()
    xp[:, :, pos] = xr.reshape(2, 128, H * W)
    return xp


def make_in_maps(inputs):
    shared = _prep(inputs)
    x = inputs['x'].astype(np.float32)
    return [{"xp": _host_pad(x[b]), **shared} for b in range(B)]


def kernel(**inputs):
    inputs = {k: np.asarray(v) for k, v in inputs.items()}
    if "nc" not in _CACHE:
        _CACHE["nc"] = _build()
    nc = _CACHE["nc"]
    in_maps = make_in_maps(inputs)
    res = run_bass_kernel_spmd(nc, in_maps, core_ids=list(range(8)))
    out = np.stack([res.results[b]["out"] for b in range(B)])
    # un-permute the deinterleaved columns: out row band k = [odd c | even c]
    out = out.reshape(B, C, H, W)
    fixed = np.empty_like(out)
    fixed[:, :, :, 1::2] = out[:, :, :, 0:32]
    fixed[:, :, :, 0::2] = out[:, :, :, 32:64]
    return fixed.astype(np.float32)
